# revision 1
# baseline (speedup 1.0000x reference)
"""Trainium2 Bass kernel for the CurvedAssociativeMemory fixed-point iteration.

Computes, for `steps` iterations:
    s <- sign(s @ (J + J^T) + h + kappa * softmax(s, axis=-1))

Strategy: data-parallel over the batch dim across 8 NeuronCores (512 rows
per core), J replicated and streamed from HBM each step.

The matmuls run in float32r (fp32 with an 11-bit stored mantissa) which the
PE processes at ~4x the fp32 rate.  Full fp32 precision is recovered with a
hi/lo split: J = Jh + Jl where Jh = fp32r(J), Jl = fp32r(J - Jh), giving
~23 mantissa bits across the pair.  The sign() applied each step makes the
iteration chaotic, so per-step precision is budgeted from a measured
noise-amplification study (flips in the final output per unit of noise
injected at step t):
  step 1 (real-valued s): 3 passes  sh@Jh + sl@Jh + sh@Jl   (eps ~2e-7)
  middle steps (s = +-1): 2 passes  s@Jh + s@Jl             (eps ~2e-7)
  final step:             1 pass    s@Jh                    (eps ~1e-4, OK
                                    because no further sign() amplifies it)
For step 1 the hi/lo split of s is precomputed on the host in transposed
layout (sT_hi persistent in SBUF, sT_lo streamed), so no on-chip
transposes are needed before the first matmuls.  Later steps transpose the
+-1 state on the PE and round it to fp32r in the PSUM->SBUF copy (exact
for +-1).  The softmax runs in fp32 with the same op sequence XLA emits.
"""

import numpy as np

N = 4096          # feature dim
B = 4096          # total batch
N_CORES = 8
B_SH = B // N_CORES   # 512 batch rows per core
P = 128               # partitions
NCHUNK = 512          # matmul moving free-dim per chunk
KO = N // P           # 32 k-tiles
NO = N // NCHUNK      # 8 n-chunks
BT = B_SH // P        # 4 batch tiles per core

# tuning knobs
JPOOL_BUFS = 8   # J-tile prefetch depth (per tag); must cover chunk boundaries
SLPOOL_BUFS = 4  # sT_lo ring depth (step 1 only)
PSUM_BUFS = 8


def fp32r_round(x):
    """Round fp32 ndarray to fp32r: RNE to 11 stored mantissa bits (top 20
    bits of the fp32 word), matching the hardware/compiler convention."""
    u = np.ascontiguousarray(x, dtype=np.float32).view(np.uint32).astype(np.uint64)
    lsb = (u >> 12) & 1
    u = (u + 0x7FF + lsb) & np.uint64(0xFFFFF000)
    return u.astype(np.uint32).view(np.float32)


def _build(steps: int, kappa: float, has_h: bool):
    import concourse.bass as bass
    import concourse.tile as tile
    import concourse.mybir as mybir
    from concourse import bacc
    from concourse.masks import make_identity

    F32 = mybir.dt.float32
    F32R = mybir.dt.float32r
    AF = mybir.ActivationFunctionType

    nc = bacc.Bacc(None)
    s_in = nc.dram_tensor("s", [B_SH, N], F32, kind="ExternalInput")
    sth_in = nc.dram_tensor("sTh", [N, B_SH], F32R, kind="ExternalInput")
    stl_in = nc.dram_tensor("sTl", [N, B_SH], F32R, kind="ExternalInput")
    jh_in = nc.dram_tensor("Jh", [N, N], F32R, kind="ExternalInput")
    jl_in = nc.dram_tensor("Jl", [N, N], F32R, kind="ExternalInput")
    h_in = nc.dram_tensor("h", [N], F32, kind="ExternalInput") if has_h else None
    out = nc.dram_tensor("out", [B_SH, N], F32, kind="ExternalOutput")

    with tile.TileContext(nc) as tc:
        with (
            tc.tile_pool(name="persist", bufs=1) as persist,
            tc.tile_pool(name="jpool", bufs=JPOOL_BUFS) as jpool,
            tc.tile_pool(name="slpool", bufs=SLPOOL_BUFS) as slpool,
            tc.tile_pool(name="scratch", bufs=2) as scratch,
            tc.tile_pool(name="stats", bufs=1) as stats,
            tc.tile_pool(name="etpool", bufs=1) as etpool,
            tc.tile_pool(name="psum", bufs=PSUM_BUFS, space="PSUM") as psum,
        ):
            ident = persist.tile([P, P], F32, tag="ident", name="ident")
            make_identity(nc, ident)

            # Initial loads go on the Scalar (ACT) DMA queue so the Sync
            # queue can start issuing the J stream immediately.
            # transposed state (fp32r): loaded from sT_hi for step 1, then
            # regenerated by PE transposes each later step
            cTh = [persist.tile([P, B_SH], F32R, tag=f"th{k}", name=f"th{k}")
                   for k in range(KO)]
            for k in range(KO):
                nc.scalar.dma_start(out=cTh[k],
                                    in_=sth_in.ap()[k * P:(k + 1) * P, :])

            # persistent state: c in natural layout, 4 tiles of [128, N]
            c = [persist.tile([P, N], F32, tag=f"c{bt}", name=f"c{bt}") for bt in range(BT)]
            for bt in range(BT):
                nc.scalar.dma_start(out=c[bt], in_=s_in.ap()[bt * P:(bt + 1) * P, :])

            h_bc = None
            if has_h:
                h_bc = persist.tile([P, N], F32, tag="hb", name="hb")
                h_ap = h_in.ap()
                nc.sync.dma_start(
                    out=h_bc,
                    in_=bass.AP(tensor=h_ap.tensor, offset=h_ap.offset,
                                ap=[[0, P], [1, N]]),
                )

            mx = [stats.tile([P, 1], F32, tag=f"mx{bt}", name=f"mx{bt}") for bt in range(BT)]
            rS = [stats.tile([P, 1], F32, tag=f"rS{bt}", name=f"rS{bt}") for bt in range(BT)]

            for t in range(steps):
                first = (t == 0)
                final = (t == steps - 1)
                # groups: (J dram, J tag, with_lo) -- with_lo adds the
                # sT_lo-stationary matmuls sharing the same streamed J tile
                if first:
                    if final:
                        groups = [(jh_in, "jh", True)]
                    else:
                        groups = [(jh_in, "jh", True), (jl_in, "jl", False)]
                elif final:
                    groups = [(jh_in, "jh", False)]
                else:
                    groups = [(jh_in, "jh", False), (jl_in, "jl", False)]

                # ---- phase A: transpose c -> cTh (not needed at t=0), stats
                if not first:
                    for k in range(KO):
                        for bt in range(BT):
                            ps_t = psum.tile([P, NCHUNK], F32, tag="pb",
                                             name="ps_t")[:, :P]
                            nc.tensor.transpose(ps_t,
                                                c[bt][:, k * P:(k + 1) * P],
                                                ident)
                            nc.vector.tensor_copy(
                                out=cTh[k][:, bt * P:(bt + 1) * P], in_=ps_t)

                for bt in range(BT):
                    et = etpool.tile([P, N], F32, tag="et", name="et")
                    nc.vector.reduce_max(out=mx[bt], in_=c[bt],
                                         axis=mybir.AxisListType.X)
                    nc.vector.tensor_scalar_sub(out=et, in0=c[bt], scalar1=mx[bt])
                    nc.scalar.activation(out=et, in_=et, func=AF.Exp)
                    ssum = stats.tile([P, 1], F32, tag="ssum", name="ssum")
                    nc.vector.reduce_sum(out=ssum, in_=et,
                                         axis=mybir.AxisListType.X)
                    # rS = kappa / sum, folding the kappa scale into the
                    # per-row scalar so the epilogue saves one full-size op
                    nc.vector.reciprocal(out=rS[bt], in_=ssum)
                    nc.scalar.mul(out=rS[bt], in_=rS[bt], mul=float(kappa))

                # ---- phase B: matmul passes + epilogue per n-chunk ----
                # k-major: all J streams for a given k are loaded together so
                # the two matmuls sharing the stationary cTh[k][bt] issue
                # back-to-back (halves effective LDWEIGHTS pressure).
                # start/stop mark the per-PSUM-tile accumulation sequence;
                # every pm_t[bt] sees the same slot order.
                n_slots = sum(KO * (2 if wl else 1) for _, _, wl in groups)
                for n in range(NO):
                    pm_t = [psum.tile([P, NCHUNK], F32, tag="pb", name="pm")
                            for _ in range(BT)]
                    slot = 0
                    for k in range(KO):
                        jts = []
                        for j_dram, j_tag, with_lo in groups:
                            jt = jpool.tile([P, NCHUNK], F32R, tag=j_tag,
                                            name="jt")
                            nc.sync.dma_start(
                                out=jt,
                                in_=j_dram.ap()[k * P:(k + 1) * P,
                                                n * NCHUNK:(n + 1) * NCHUNK])
                            jts.append(jt)
                        with_lo = groups[0][2]
                        if with_lo:
                            slt = slpool.tile([P, B_SH], F32R, tag="sl",
                                              name="slt")
                            nc.sync.dma_start(
                                out=slt,
                                in_=stl_in.ap()[k * P:(k + 1) * P, :])
                        k_slots = len(jts) + (1 if with_lo else 0)
                        for bt in range(BT):
                            bsl = slice(bt * P, (bt + 1) * P)
                            sl_i = slot
                            for jt in jts:
                                nc.tensor.matmul(
                                    pm_t[bt], cTh[k][:, bsl], jt,
                                    start=(sl_i == 0),
                                    stop=(sl_i == n_slots - 1))
                                sl_i += 1
                            if with_lo:
                                nc.tensor.matmul(
                                    pm_t[bt], slt[:, bsl], jts[0],
                                    start=False,
                                    stop=(sl_i == n_slots - 1))
                                sl_i += 1
                        slot += k_slots
                    nsl = slice(n * NCHUNK, (n + 1) * NCHUNK)
                    for bt in range(BT):
                        m_sl = pm_t[bt]
                        u = scratch.tile([P, NCHUNK], F32, tag="u", name="u")
                        if has_h:
                            nc.vector.tensor_add(out=u, in0=m_sl, in1=h_bc[:, nsl])
                        q = scratch.tile([P, NCHUNK], F32, tag="q", name="q")
                        nc.vector.tensor_scalar_sub(out=q, in0=c[bt][:, nsl],
                                                    scalar1=mx[bt])
                        nc.scalar.activation(out=q, in_=q, func=AF.Exp)
                        nc.vector.tensor_scalar_mul(out=q, in0=q, scalar1=rS[bt])
                        if has_h:
                            nc.vector.tensor_add(out=u, in0=u, in1=q)
                        else:
                            nc.vector.tensor_add(out=u, in0=m_sl, in1=q)
                        if final:
                            # stream the final sign straight to DRAM per chunk
                            # instead of one big copy at the end
                            w = scratch.tile([P, NCHUNK], F32, tag="w", name="w")
                            nc.scalar.activation(out=w, in_=u, func=AF.Sign)
                            nc.scalar.dma_start(
                                out=out.ap()[bt * P:(bt + 1) * P, nsl], in_=w)
                        else:
                            nc.scalar.activation(out=c[bt][:, nsl], in_=u,
                                                 func=AF.Sign)

    nc.finalize()
    return nc


LAST_RESULTS = None  # BassKernelResults from the most recent kernel() call
LAST_NC = None       # finalized Bass module from the most recent kernel() call


def kernel(s, J, h, kappa, steps):
    import os
    from concourse.bass_utils import run_bass_kernel_spmd

    s = np.ascontiguousarray(np.asarray(s, dtype=np.float32))
    J = np.asarray(J, dtype=np.float32)
    h = np.asarray(h, dtype=np.float32)
    kappa_f = float(np.asarray(kappa))
    steps_i = int(np.asarray(steps))

    Jsym = np.ascontiguousarray(J + J.T)
    Jh = fp32r_round(Jsym)
    Jl = np.ascontiguousarray(fp32r_round(Jsym - Jh))
    Jh = np.ascontiguousarray(Jh)
    has_h = bool(np.any(h))

    nc = _build(steps_i, kappa_f, has_h)
    global LAST_NC
    LAST_NC = nc

    in_maps = []
    for i in range(N_CORES):
        s_sh = np.ascontiguousarray(s[i * B_SH:(i + 1) * B_SH])
        sh = fp32r_round(s_sh)
        sl = fp32r_round(s_sh - sh)
        m = {"s": s_sh,
             "sTh": np.ascontiguousarray(sh.T),
             "sTl": np.ascontiguousarray(sl.T),
             "Jh": Jh, "Jl": Jl}
        if has_h:
            m["h"] = h
        in_maps.append(m)

    trace = os.environ.get("CAM_TRACE", "") == "1"
    res = run_bass_kernel_spmd(nc, in_maps, core_ids=list(range(N_CORES)),
                               trace=trace)
    global LAST_RESULTS
    LAST_RESULTS = res
    out = np.concatenate([r["out"] for r in res.results], axis=0)
    return out.astype(np.float32, copy=False)


if __name__ == "__main__":
    rng = np.random.default_rng(0)
    s = rng.standard_normal((B, N)).astype(np.float32)
    J0 = (0.01 * rng.standard_normal((N, N))).astype(np.float32)
    J = ((J0 + J0.T) / 2).astype(np.float32)
    out = kernel(s=s, J=J, h=np.zeros(N, np.float32),
                 kappa=np.float32(0.2), steps=3)
    print(out.shape, np.unique(out, return_counts=True))

